# revision 1
# baseline (speedup 1.0000x reference)
"""FP32 -> FP8 E4M3 bit-pulse converter on 8 Trainium2 NeuronCores.

Input : fp32_pulse [2097152, 32] float32 of 0/1 pulses, [S, E7..E0, M22..M0]
Output: [2097152, 8] float32 of 0/1 pulses, [S, E3..E0, M2..M0]

Strategy (per core, batch-sharded 8 ways):
  - q = 32*exp + 16*m22 + 8*m21 + 4*m20 + 2*m19 + sticky  (13-bit int) via a
    scalar_tensor_tensor MAC chain; sticky = OR (reduce_max) of the 19 low
    mantissa bit pulses.
  - v = bitcast(int32(max(q,3712) * 2^18)): exactly the fp32 with exponent
    field = exp, mantissa = m22..m19 | sticky<<18.  The underflow clamp maps
    exp<=116 to a value that rounds to zero in fp8.
  - Hardware fp32->fp8e4 copy performs the exact RNE (incl. subnormals).
  - Overflow (exp>=135) forces byte 0x7E (=15/6) per the reference.
  - 7 low bits of the byte + the sign pulse are scattered to the output.
"""

import numpy as np

import concourse.bass as bass
import concourse.bacc as bacc
import concourse.mybir as mybir
from concourse import tile
from concourse.bass_utils import run_bass_kernel_spmd

N_ROWS = 2097152
N_CORES = 8
ROWS_PER_CORE = N_ROWS // N_CORES          # 262144
F32 = 8192                                  # fp32 elems per partition per chunk

dt = mybir.dt
Alu = mybir.AluOpType

MAC_W = [float(2 ** (13 - i)) for i in range(1, 13)]  # col i weight, i=1..12


def _build_program(repeat: int = 1, rows_per_core: int = ROWS_PER_CORE,
                   f32: int = F32, compute_only: bool = False):
    seg = f32 // 32                         # rows per partition per chunk
    rows_per_chunk = 128 * seg
    n_chunks = rows_per_core // rows_per_chunk
    fo = seg * 8
    assert n_chunks * rows_per_chunk == rows_per_core

    nc = bacc.Bacc("TRN2", target_bir_lowering=False, debug=False,
                   num_devices=N_CORES)
    x_dram = nc.dram_tensor("x", [rows_per_core, 32], dt.float32,
                            kind="ExternalInput")
    y_dram = nc.dram_tensor("y", [rows_per_core, 8], dt.float32,
                            kind="ExternalOutput")
    x_ap = x_dram.ap().rearrange("(c p f) w -> c p (f w)", c=n_chunks, p=128)
    y_ap = y_dram.ap().rearrange("(c p f) w -> c p (f w)", c=n_chunks, p=128)

    with tile.TileContext(nc) as tc:
        with (
            tc.tile_pool(name="xin", bufs=3) as xin_pool,
            tc.tile_pool(name="out", bufs=3) as out_pool,
            tc.tile_pool(name="wrk", bufs=3) as wrk,
        ):
            xf0 = None
            for c in [c for _ in range(repeat) for c in range(n_chunks)]:
                if compute_only:
                    if xf0 is None:
                        xf0 = xin_pool.tile([128, f32], dt.float32, tag="xf")
                        nc.sync.dma_start(xf0[:], x_ap[0])
                    xf = xf0
                else:
                    xf = xin_pool.tile([128, f32], dt.float32, tag="xf")
                    nc.sync.dma_start(xf[:], x_ap[c])

                x3d = xf[:].rearrange("p (s c) -> p s c", c=32)

                # sticky = OR of cols 13..31 (0/1 pulses -> reduce_max)
                red = wrk.tile([128, seg], dt.float32, tag="red")
                nc.vector.tensor_reduce(red[:], x3d[:, :, 13:32],
                                        axis=mybir.AxisListType.X, op=Alu.max)

                # q via windowed Horner tree (cols 1..12):
                #   P_k = 2*c(2k-1) + c(2k)   (6-wide, one op)
                #   Q_k = 4*P(2k-1) + P(2k)   (3-wide, one op)
                #   R1 = 16*Q1 + Q2; T = 16*R1 + Q3; q = 2*T + sticky
                y1 = wrk.tile([128, seg * 6], dt.float32, tag="y1")
                y1_3d = y1[:].rearrange("p (s k) -> p s k", k=6)
                nc.vector.scalar_tensor_tensor(y1_3d[:], x3d[:, :, 1:12:2],
                                               2.0, x3d[:, :, 2:13:2],
                                               op0=Alu.mult, op1=Alu.add)
                y2 = wrk.tile([128, seg * 3], dt.float32, tag="y2")
                y2_3d = y2[:].rearrange("p (s k) -> p s k", k=3)
                nc.vector.scalar_tensor_tensor(y2_3d[:], y1_3d[:, :, 0::2],
                                               4.0, y1_3d[:, :, 1::2],
                                               op0=Alu.mult, op1=Alu.add)
                r1 = wrk.tile([128, seg], dt.float32, tag="r1")
                nc.vector.scalar_tensor_tensor(r1[:], y2_3d[:, :, 0], 16.0,
                                               y2_3d[:, :, 1],
                                               op0=Alu.mult, op1=Alu.add)
                t_t = wrk.tile([128, seg], dt.float32, tag="t")
                nc.vector.scalar_tensor_tensor(t_t[:], r1[:], 16.0,
                                               y2_3d[:, :, 2],
                                               op0=Alu.mult, op1=Alu.add)
                q = wrk.tile([128, seg], dt.float32, tag="q")
                nc.vector.scalar_tensor_tensor(q[:], t_t[:], 2.0, red[:],
                                               op0=Alu.mult, op1=Alu.add)

                # v bits = int32(max(q, 3712) * 2^18); fp8 cast on ACT
                vb = wrk.tile([128, seg], dt.int32, tag="vb")
                nc.vector.tensor_scalar(vb[:], q[:], 3712.0, 262144.0,
                                        op0=Alu.max, op1=Alu.mult)
                f8 = wrk.tile([128, seg], dt.float8e4, tag="f8")
                nc.scalar.copy(f8[:], vb[:].bitcast(dt.float32))
                # overflow select in the byte domain:
                #   uf = min(u,126) | ((q>=4320)*6)   (in-range bytes <= 120)
                u1 = wrk.tile([128, seg], dt.int8, tag="u1")
                nc.vector.tensor_scalar(u1[:], f8[:].bitcast(dt.uint8), 126,
                                        None, op0=Alu.min)
                t6 = wrk.tile([128, seg], dt.int8, tag="t6")
                nc.vector.tensor_scalar(t6[:], q[:], 4320.0, 6.0,
                                        op0=Alu.is_ge, op1=Alu.mult)
                ui = wrk.tile([128, seg], dt.int8, tag="ui")
                nc.vector.tensor_tensor(ui[:], u1[:], t6[:], op=Alu.bitwise_or)

                o_i = wrk.tile([128, fo], dt.int8, tag="oi")
                oi3d = o_i[:].rearrange("p (s c) -> p s c", c=8)
                nc.scalar.copy(oi3d[:, :, 0], x3d[:, :, 0])      # sign
                for j in range(1, 8):
                    nc.vector.tensor_scalar(oi3d[:, :, j], ui[:], 7 - j, 1,
                                            op0=Alu.logical_shift_right,
                                            op1=Alu.bitwise_and)
                o_t = out_pool.tile([128, fo], dt.float32, tag="o")
                nc.scalar.copy(o_t[:], o_i[:])

                if not compute_only or c == n_chunks - 1:
                    nc.scalar.dma_start(y_ap[c], o_t[:])

    nc.compile()
    return nc


_NC_CACHE = {}


def _get_nc(repeat: int = 1):
    if repeat not in _NC_CACHE:
        _NC_CACHE[repeat] = _build_program(repeat)
    return _NC_CACHE[repeat]


def run(fp32_pulse: np.ndarray, trace: bool = False):
    fp32_pulse = np.ascontiguousarray(np.asarray(fp32_pulse, dtype=np.float32))
    assert fp32_pulse.shape == (N_ROWS, 32), fp32_pulse.shape
    nc = _get_nc()
    shards = np.split(fp32_pulse, N_CORES, axis=0)
    in_maps = [{"x": s} for s in shards]
    res = run_bass_kernel_spmd(nc, in_maps, list(range(N_CORES)), trace=trace)
    out = np.concatenate([r["y"] for r in res.results], axis=0)
    return out.astype(np.float32, copy=False), res


def kernel(fp32_pulse: np.ndarray) -> np.ndarray:
    out, _ = run(fp32_pulse, trace=False)
    return out



# revision 2
# speedup vs baseline: 1.1031x; 1.1031x over previous
"""FP32 -> FP8 E4M3 bit-pulse converter on 8 Trainium2 NeuronCores.

Input : fp32_pulse [2097152, 32] float32 of 0/1 pulses, [S, E7..E0, M22..M0]
Output: [2097152, 8] float32 of 0/1 pulses, [S, E3..E0, M2..M0]

Design: device emits ONE packed fp8 byte per row (S<<7|E<<3|M); host expands
with np.unpackbits.  Per-core traffic 33.55 MB in + 0.26 MB out ==
input-read roofline (~94 us at 358 GB/s/core).  The chunk schedule is
TAPERED: large chunks first (DMA efficiency), tiny chunks last so the
final chunk's compute tail (the only compute not hidden under DMA) is
~1 us.  All tiles are allocated per-iteration in the same scope (no
cross-scope accumulator - that breaks Tile dependency tracking).

Math per chunk (rows batch-sharded 8 ways, row r = p*2048 + j on a core):
  - q = 32*exp + 16*m22 + 8*m21 + 4*m20 + 2*m19 + sticky  (13-bit int) via
    scalar_tensor_tensor MAC tree; sticky = OR (reduce_max) of the 19 low
    mantissa pulses.
  - v = bitcast(int32(max(q,3712) * 2^18)): the fp32 with exponent field =
    exp, mantissa = m22..m19 | sticky<<18; exp<=116 clamps to a value that
    rounds to 0 in fp8.
  - fp32->fp8e4 copy on ACT does exact RNE (incl. subnormals).
  - overflow (exp>=135): 7-bit field forced to 0x7E via min(u,126)|6.
  - byte = (sign<<7) | field, one uint8 DMA per chunk.
"""

import numpy as np

import concourse.bass as bass
import concourse.bacc as bacc
import concourse.mybir as mybir
from concourse import tile
from concourse.bass_utils import run_bass_kernel_spmd

N_ROWS = 2097152
N_CORES = 8
ROWS_PER_CORE = N_ROWS // N_CORES          # 262144
JPP = ROWS_PER_CORE // 128                 # rows per partition = 2048

# chunk schedule: rows-per-partition per chunk, sums to JPP; big chunks for
# DMA efficiency, one small tail chunk so the un-hidden final compute is short
SEGS = [256] * 7 + [128, 64, 64]

dt = mybir.dt
Alu = mybir.AluOpType


def _build_program(repeat: int = 1, segs=None, jpp: int = JPP,
                   compute_only: bool = False, dma_only: bool = False,
                   xin_bufs: int = 3, wrk_bufs: int = 3, out_bufs: int = 3,
                   dma_engines=("sync",)):
    segs = list(SEGS) if segs is None else list(segs)
    assert sum(segs) == jpp
    smax = max(segs)

    nc = bacc.Bacc("TRN2", target_bir_lowering=False, debug=False,
                   num_devices=N_CORES)
    x_dram = nc.dram_tensor("x", [128 * jpp, 32], dt.float32,
                            kind="ExternalInput")
    y_dram = nc.dram_tensor("y", [128, jpp], dt.uint8,
                            kind="ExternalOutput")
    # row r of this core's shard is (p, j) with r = p*jpp + j
    x_ap = x_dram.ap().rearrange("(p j) w -> p (j w)", p=128)
    y_ap = y_dram.ap()

    with tile.TileContext(nc) as tc:
        with (
            tc.tile_pool(name="xin", bufs=xin_bufs) as xin_pool,
            tc.tile_pool(name="out", bufs=out_bufs) as out_pool,
            tc.tile_pool(name="wrk", bufs=wrk_bufs) as wrk,
        ):
            xf0 = None
            for r in range(repeat):
                o = 0
                for ci, s in enumerate(segs):
                    f32 = s * 32
                    eng = getattr(nc, dma_engines[ci % len(dma_engines)])
                    if compute_only:
                        if xf0 is None:
                            xf0 = xin_pool.tile([128, smax * 32], dt.float32,
                                                tag="xf")
                            nc.sync.dma_start(xf0[:, :f32],
                                              x_ap[:, o * 32:(o + s) * 32])
                        xt = xf0
                    else:
                        xt = xin_pool.tile([128, smax * 32], dt.float32,
                                           tag="xf")
                        eng.dma_start(xt[:, :f32],
                                      x_ap[:, o * 32:(o + s) * 32])

                    if dma_only:
                        o += s
                        continue

                    x3d = xt[:, :f32].rearrange("p (s c) -> p s c", c=32)

                    # sticky = OR of cols 13..31 (0/1 pulses -> reduce_max)
                    red = wrk.tile([128, smax], dt.float32, tag="red")
                    nc.vector.tensor_reduce(red[:, :s], x3d[:, :, 13:32],
                                            axis=mybir.AxisListType.X,
                                            op=Alu.max)

                    # q via windowed Horner tree (cols 1..12):
                    #   P_k = 2*c(2k-1) + c(2k); Q_k = 4*P(2k-1) + P(2k)
                    #   R1 = 16*Q1 + Q2; T = 16*R1 + Q3; q = 2*T + sticky
                    y1 = wrk.tile([128, smax * 6], dt.float32, tag="y1")
                    y1_3d = y1[:, :s * 6].rearrange("p (s k) -> p s k", k=6)
                    nc.vector.scalar_tensor_tensor(y1_3d[:], x3d[:, :, 1:12:2],
                                                   2.0, x3d[:, :, 2:13:2],
                                                   op0=Alu.mult, op1=Alu.add)
                    y2 = wrk.tile([128, smax * 3], dt.float32, tag="y2")
                    y2_3d = y2[:, :s * 3].rearrange("p (s k) -> p s k", k=3)
                    nc.vector.scalar_tensor_tensor(y2_3d[:], y1_3d[:, :, 0::2],
                                                   4.0, y1_3d[:, :, 1::2],
                                                   op0=Alu.mult, op1=Alu.add)
                    r1 = wrk.tile([128, smax], dt.float32, tag="r1")
                    nc.vector.scalar_tensor_tensor(r1[:, :s], y2_3d[:, :, 0],
                                                   16.0, y2_3d[:, :, 1],
                                                   op0=Alu.mult, op1=Alu.add)
                    t_t = wrk.tile([128, smax], dt.float32, tag="t")
                    nc.vector.scalar_tensor_tensor(t_t[:, :s], r1[:, :s], 16.0,
                                                   y2_3d[:, :, 2],
                                                   op0=Alu.mult, op1=Alu.add)
                    q = wrk.tile([128, smax], dt.float32, tag="q")
                    nc.vector.scalar_tensor_tensor(q[:, :s], t_t[:, :s], 2.0,
                                                   red[:, :s],
                                                   op0=Alu.mult, op1=Alu.add)

                    # v bits = int32(max(q, 3712) * 2^18); fp8 cast on ACT
                    vb = wrk.tile([128, smax], dt.int32, tag="vb")
                    nc.vector.tensor_scalar(vb[:, :s], q[:, :s], 3712.0,
                                            262144.0, op0=Alu.max,
                                            op1=Alu.mult)
                    f8 = wrk.tile([128, smax], dt.float8e4, tag="f8")
                    nc.scalar.copy(f8[:, :s], vb[:, :s].bitcast(dt.float32))
                    # byte = min(u,126) | ((q>=4320)*6) | (sign<<7):
                    # overflow forces the 7-bit field to 0x7E; sign<<7 on ACT
                    u1 = wrk.tile([128, smax], dt.uint8, tag="u1")
                    nc.vector.tensor_scalar(u1[:, :s],
                                            f8[:, :s].bitcast(dt.uint8),
                                            126, None, op0=Alu.min)
                    t6 = wrk.tile([128, smax], dt.uint8, tag="t6")
                    nc.vector.tensor_scalar(t6[:, :s], q[:, :s], 4320.0, 6.0,
                                            op0=Alu.is_ge, op1=Alu.mult)
                    s128 = wrk.tile([128, smax], dt.uint8, tag="s128")
                    nc.scalar.mul(s128[:, :s], x3d[:, :, 0], 128.0)
                    sor = wrk.tile([128, smax], dt.uint8, tag="sor")
                    nc.vector.tensor_tensor(sor[:, :s], t6[:, :s],
                                            s128[:, :s], op=Alu.bitwise_or)
                    oc = out_pool.tile([128, smax], dt.uint8, tag="oc")
                    nc.vector.tensor_tensor(oc[:, :s], u1[:, :s], sor[:, :s],
                                            op=Alu.bitwise_or)

                    if not compute_only or r == repeat - 1:
                        nc.scalar.dma_start(y_ap[:, o:o + s], oc[:, :s])
                    o += s

    nc.compile()
    return nc


_NC_CACHE = {}


def _get_nc(repeat: int = 1):
    if repeat not in _NC_CACHE:
        _NC_CACHE[repeat] = _build_program(repeat)
    return _NC_CACHE[repeat]


def run(fp32_pulse: np.ndarray, trace: bool = False):
    fp32_pulse = np.ascontiguousarray(np.asarray(fp32_pulse, dtype=np.float32))
    assert fp32_pulse.shape == (N_ROWS, 32), fp32_pulse.shape
    nc = _get_nc()
    shards = np.split(fp32_pulse, N_CORES, axis=0)
    in_maps = [{"x": s} for s in shards]
    res = run_bass_kernel_spmd(nc, in_maps, list(range(N_CORES)), trace=trace)
    # y[p, j] is the byte for shard row p*jpp + j == flat row order
    packed = np.concatenate([r["y"].reshape(-1) for r in res.results])
    out = np.unpackbits(packed[:, None], axis=1)
    return out.astype(np.float32), res


def kernel(fp32_pulse: np.ndarray) -> np.ndarray:
    out, _ = run(fp32_pulse, trace=False)
    return out


# revision 3
# speedup vs baseline: 1.3800x; 1.2510x over previous
"""FP32 -> FP8 E4M3 bit-pulse converter on 8 Trainium2 NeuronCores.

Input : fp32_pulse [2097152, 32] float32 of 0/1 pulses, [S, E7..E0, M22..M0]
Output: [2097152, 8] float32 of 0/1 pulses, [S, E3..E0, M2..M0]

Design: device emits ONE packed fp8 byte per row (S<<7|E<<3|M); host expands
with np.unpackbits.  Per-core traffic 33.55 MB in + 0.26 MB out ==
input-read roofline (~94 us at 358 GB/s/core).  The chunk schedule is
TAPERED: large chunks first (DMA efficiency), tiny chunks last so the
final chunk's compute tail (the only compute not hidden under DMA) is
~1 us.  All tiles are allocated per-iteration in the same scope (no
cross-scope accumulator - that breaks Tile dependency tracking).

Math per chunk (rows batch-sharded 8 ways, row r = p*2048 + j on a core):
  - q = 32*exp + 16*m22 + 8*m21 + 4*m20 + 2*m19 + sticky  (13-bit int) via
    scalar_tensor_tensor MAC tree; sticky = OR (reduce_max) of the 19 low
    mantissa pulses.
  - v = bitcast(int32(max(q,3712) * 2^18)): the fp32 with exponent field =
    exp, mantissa = m22..m19 | sticky<<18; exp<=116 clamps to a value that
    rounds to 0 in fp8.
  - fp32->fp8e4 copy on ACT does exact RNE (incl. subnormals).
  - overflow (exp>=135): 7-bit field forced to 0x7E via min(u,126)|6.
  - byte = (sign<<7) | field, one uint8 DMA per chunk.
"""

import numpy as np

import concourse.bass as bass
import concourse.bacc as bacc
import concourse.mybir as mybir
from concourse import tile
from concourse.bass_utils import run_bass_kernel_spmd

N_ROWS = 2097152
N_CORES = 8
ROWS_PER_CORE = N_ROWS // N_CORES          # 262144
JPP = ROWS_PER_CORE // 128                 # rows per partition = 2048

# chunk schedule: rows-per-partition per chunk, sums to JPP; big chunks for
# DMA efficiency, one small tail chunk so the un-hidden final compute is short
SEGS = [320] * 6 + [64, 64]

dt = mybir.dt
Alu = mybir.AluOpType


def _build_program(repeat: int = 1, segs=None, jpp: int = JPP,
                   compute_only: bool = False, dma_only: bool = False,
                   xin_bufs: int = 4, wrk_bufs: int = 2, out_bufs: int = 3,
                   dma_engines=("sync",)):
    segs = list(SEGS) if segs is None else list(segs)
    assert sum(segs) == jpp
    smax = max(segs)

    nc = bacc.Bacc("TRN2", target_bir_lowering=False, debug=False,
                   num_devices=N_CORES)
    x_dram = nc.dram_tensor("x", [128 * jpp, 32], dt.float32,
                            kind="ExternalInput")
    y_dram = nc.dram_tensor("y", [128, jpp], dt.uint8,
                            kind="ExternalOutput")
    # row r of this core's shard is (p, j) with r = p*jpp + j
    x_ap = x_dram.ap().rearrange("(p j) w -> p (j w)", p=128)
    y_ap = y_dram.ap()

    with tile.TileContext(nc) as tc:
        with (
            tc.tile_pool(name="xin", bufs=xin_bufs) as xin_pool,
            tc.tile_pool(name="out", bufs=out_bufs) as out_pool,
            tc.tile_pool(name="wrk", bufs=wrk_bufs) as wrk,
        ):
            xf0 = None
            for r in range(repeat):
                o = 0
                for ci, s in enumerate(segs):
                    f32 = s * 32
                    eng = getattr(nc, dma_engines[ci % len(dma_engines)])
                    if compute_only:
                        if xf0 is None:
                            xf0 = xin_pool.tile([128, smax * 32], dt.float32,
                                                tag="xf")
                            nc.sync.dma_start(xf0[:, :f32],
                                              x_ap[:, o * 32:(o + s) * 32])
                        xt = xf0
                    else:
                        xt = xin_pool.tile([128, smax * 32], dt.float32,
                                           tag="xf")
                        eng.dma_start(xt[:, :f32],
                                      x_ap[:, o * 32:(o + s) * 32])

                    if dma_only:
                        o += s
                        continue

                    x3d = xt[:, :f32].rearrange("p (s c) -> p s c", c=32)

                    # sticky = OR of cols 13..31 (0/1 pulses -> reduce_max)
                    red = wrk.tile([128, smax], dt.float32, tag="red")
                    nc.vector.tensor_reduce(red[:, :s], x3d[:, :, 13:32],
                                            axis=mybir.AxisListType.X,
                                            op=Alu.max)

                    # q via windowed Horner tree (cols 1..12):
                    #   P_k = 2*c(2k-1) + c(2k); Q_k = 4*P(2k-1) + P(2k)
                    #   R1 = 16*Q1 + Q2; T = 16*R1 + Q3; q = 2*T + sticky
                    y1 = wrk.tile([128, smax * 6], dt.float32, tag="y1")
                    y1_3d = y1[:, :s * 6].rearrange("p (s k) -> p s k", k=6)
                    nc.vector.scalar_tensor_tensor(y1_3d[:], x3d[:, :, 1:12:2],
                                                   2.0, x3d[:, :, 2:13:2],
                                                   op0=Alu.mult, op1=Alu.add)
                    y2 = wrk.tile([128, smax * 3], dt.float32, tag="y2")
                    y2_3d = y2[:, :s * 3].rearrange("p (s k) -> p s k", k=3)
                    nc.vector.scalar_tensor_tensor(y2_3d[:], y1_3d[:, :, 0::2],
                                                   4.0, y1_3d[:, :, 1::2],
                                                   op0=Alu.mult, op1=Alu.add)
                    r1 = wrk.tile([128, smax], dt.float32, tag="r1")
                    nc.vector.scalar_tensor_tensor(r1[:, :s], y2_3d[:, :, 0],
                                                   16.0, y2_3d[:, :, 1],
                                                   op0=Alu.mult, op1=Alu.add)
                    t_t = wrk.tile([128, smax], dt.float32, tag="t")
                    nc.vector.scalar_tensor_tensor(t_t[:, :s], r1[:, :s], 16.0,
                                                   y2_3d[:, :, 2],
                                                   op0=Alu.mult, op1=Alu.add)
                    q = wrk.tile([128, smax], dt.float32, tag="q")
                    nc.vector.scalar_tensor_tensor(q[:, :s], t_t[:, :s], 2.0,
                                                   red[:, :s],
                                                   op0=Alu.mult, op1=Alu.add)

                    # v bits = int32(max(q, 3712) * 2^18); fp8 cast on ACT
                    vb = wrk.tile([128, smax], dt.int32, tag="vb")
                    nc.vector.tensor_scalar(vb[:, :s], q[:, :s], 3712.0,
                                            262144.0, op0=Alu.max,
                                            op1=Alu.mult)
                    f8 = wrk.tile([128, smax], dt.float8e4, tag="f8")
                    nc.scalar.copy(f8[:, :s], vb[:, :s].bitcast(dt.float32))
                    # byte = min(u,126) | ((q>=4320)*6) | (sign<<7):
                    # overflow forces the 7-bit field to 0x7E; sign<<7 on ACT
                    u1 = wrk.tile([128, smax], dt.uint8, tag="u1")
                    nc.vector.tensor_scalar(u1[:, :s],
                                            f8[:, :s].bitcast(dt.uint8),
                                            126, None, op0=Alu.min)
                    t6 = wrk.tile([128, smax], dt.uint8, tag="t6")
                    nc.vector.tensor_scalar(t6[:, :s], q[:, :s], 4320.0, 6.0,
                                            op0=Alu.is_ge, op1=Alu.mult)
                    s128 = wrk.tile([128, smax], dt.uint8, tag="s128")
                    nc.scalar.mul(s128[:, :s], x3d[:, :, 0], 128.0)
                    sor = wrk.tile([128, smax], dt.uint8, tag="sor")
                    nc.vector.tensor_tensor(sor[:, :s], t6[:, :s],
                                            s128[:, :s], op=Alu.bitwise_or)
                    oc = out_pool.tile([128, smax], dt.uint8, tag="oc")
                    nc.vector.tensor_tensor(oc[:, :s], u1[:, :s], sor[:, :s],
                                            op=Alu.bitwise_or)

                    if not compute_only or r == repeat - 1:
                        nc.scalar.dma_start(y_ap[:, o:o + s], oc[:, :s])
                    o += s

    nc.compile()
    return nc


_NC_CACHE = {}


def _get_nc(repeat: int = 1):
    if repeat not in _NC_CACHE:
        _NC_CACHE[repeat] = _build_program(repeat)
    return _NC_CACHE[repeat]


def run(fp32_pulse: np.ndarray, trace: bool = False):
    fp32_pulse = np.ascontiguousarray(np.asarray(fp32_pulse, dtype=np.float32))
    assert fp32_pulse.shape == (N_ROWS, 32), fp32_pulse.shape
    nc = _get_nc()
    shards = np.split(fp32_pulse, N_CORES, axis=0)
    in_maps = [{"x": s} for s in shards]
    res = run_bass_kernel_spmd(nc, in_maps, list(range(N_CORES)), trace=trace)
    # y[p, j] is the byte for shard row p*jpp + j == flat row order
    packed = np.concatenate([r["y"].reshape(-1) for r in res.results])
    out = np.unpackbits(packed[:, None], axis=1)
    return out.astype(np.float32), res


def kernel(fp32_pulse: np.ndarray) -> np.ndarray:
    out, _ = run(fp32_pulse, trace=False)
    return out
